# revision 1
# baseline (speedup 1.0000x reference)
"""Gated linear attention (GLA) Bass kernel for Trainium2, 8 NeuronCores.

Sharding: one core per (batch, head) pair -- B=2 x H=4 = 8 cores.
Each core computes its head's full pipeline with a chunked-parallel form of
the gated recurrence (chunk = 128), entirely on-device:

  z   = x @ (Wgk1@Wgk2)          (fused on host)
  sp  = softplus(-(z + bgk2))    = -log_sigmoid(z + bgk2)
  G   = -cumsum_per_chunk(sp)/16 (inclusive)
  qt  = (x @ Wq)^T * exp(G)*scale ; kt = (x @ Wk)^T * exp(-G)
  per chunk c:  AT = kt^T-block' qt-block  (masked s<=t)
                O  = AT^T @ V + qt^T @ S ;  S = (S + k~^T V) * exp(G_last)
  o   = O * rsqrt(mean(O^2)+eps) * (g*sigmoid(g))   [g = x @ Wg]
  out = o @ ((gnorm*Wo_head) @ Whead)               (fused on host)

Host gathers: out[b] = sum_h core_out[b,h] + bhead.
"""
import sys, os
sys.path.insert(0, "/opt/trn_rl_repo")

import numpy as np

B, T, D = 2, 2048, 512
H = 4
dk, dv = 64, 128          # per-head key/value dims
C = 128                   # chunk length
GATE_NORM = 16.0
EPS = 1e-5
SCALE = dk ** -0.5

_CACHE = {}
BF16_CHUNK = False  # bf16 chunk matmuls: 141us vs 151us but 3.5e-3 err - rejected


def build(t=T):
    import concourse.bass as bass  # noqa: F401
    from concourse import bacc, mybir
    import concourse.tile as tile
    import concourse.hw_specs as hw_specs

    F32 = mybir.dt.float32
    F32R = mybir.dt.float32r
    BF16 = mybir.dt.bfloat16
    AF = mybir.ActivationFunctionType
    OP = mybir.AluOpType
    bf = BF16_CHUNK

    # All activation funcs used here (Exp, Ln, Square, Copy, Identity) live
    # together in the natural_log_exp_and_others table, but the table chooser
    # assigns each func to the first table containing it (Exp -> exp_and_others,
    # Ln -> natural_log), which thrashes ACT_TABLE_LOADs between every Ln/Exp
    # pair (measured 41 loads, 52us).  Steer the chooser by removing our funcs
    # from every other table in the cached table dict (indices are preserved,
    # so act_func_set_id stays aligned with the compiler's act_info.json).
    need = {AF.Exp, AF.Ln, AF.Square, AF.Copy, AF.Identity}
    keep = "natural_log_exp_and_others"
    tabs = hw_specs.get_activation_tables("gen3")
    if keep in tabs and need <= tabs[keep]:
        for name, s in tabs.items():
            if name != keep:
                s -= need

    nch = t // C              # chunks
    nts = t // 512            # 512-wide time slices
    assert t % 512 == 0

    nc = bacc.Bacc("TRN2", target_bir_lowering=False, debug=False)

    xt_d = nc.dram_tensor("xt", [128, 4, t], F32R, kind="ExternalInput")
    wqk_d = nc.dram_tensor("wqk", [128, 4, 2 * dk], F32R, kind="ExternalInput")
    wvg_d = nc.dram_tensor("wvg", [128, 4, 2 * dv], F32R, kind="ExternalInput")
    wgk_d = nc.dram_tensor("wgk12", [128, 4, dk], F32R, kind="ExternalInput")
    wf_d = nc.dram_tensor("wfused", [dv, 10], F32, kind="ExternalInput")
    nb_d = nc.dram_tensor("nbgk2", [dk, 1], F32, kind="ExternalInput")
    um_d = nc.dram_tensor("umask", [C, C], F32, kind="ExternalInput")
    id_d = nc.dram_tensor("ident", [128, 128], F32, kind="ExternalInput")
    out_d = nc.dram_tensor("out10", [t, 10], F32, kind="ExternalOutput")

    with tile.TileContext(nc) as tc:
        with (
            tc.tile_pool(name="wt", bufs=1) as wt,
            tc.tile_pool(name="big", bufs=1) as big,
            tc.tile_pool(name="sm", bufs=3) as sm,
            tc.tile_pool(name="ck", bufs=5) as ck,
            tc.tile_pool(name="pp", bufs=4, space="PSUM") as pp,
            tc.tile_pool(name="pc", bufs=4, space="PSUM") as pc,
        ):
            # ---- weights / consts (small, gpsimd queue) ----
            wqk_sb = wt.tile([128, 4, 2 * dk], F32R)
            wvg_sb = wt.tile([128, 4, 2 * dv], F32R)
            wgk_sb = wt.tile([128, 4, dk], F32R)
            wf_sb = wt.tile([dv, 10], F32)
            nb_sb = wt.tile([dk, 1], F32)
            um_sb = wt.tile([C, C], F32)
            id_sb = wt.tile([128, 128], F32)
            # matmul weights on the fast sync queue ahead of the x^T stream;
            # small consts on the gpsimd queue in parallel
            nc.sync.dma_start(wgk_sb[:], wgk_d[:])
            nc.sync.dma_start(wqk_sb[:], wqk_d[:])
            nc.sync.dma_start(wvg_sb[:], wvg_d[:])
            nc.gpsimd.dma_start(wf_sb[:], wf_d[:])
            nc.gpsimd.dma_start(nb_sb[:], nb_d[:])
            nc.gpsimd.dma_start(um_sb[:], um_d[:])
            nc.gpsimd.dma_start(id_sb[:], id_d[:])
            eps_sb = wt.tile([128, 1], F32)
            nc.vector.memset(eps_sb[:], EPS)
            lnsc_sb = wt.tile([dk, 1], F32)
            nc.vector.memset(lnsc_sb[:], float(np.log(SCALE)))
            if bf:
                idb_sb = wt.tile([128, 128], BF16)
                nc.vector.tensor_copy(idb_sb[:], id_sb[:])

            # ---- big SBUF tensors ----
            xT = big.tile([128, 4, t], F32R)      # x^T per 128-d-chunk
            qt = big.tile([dk, t], F32)           # q-tilde transposed
            kt = big.tile([dk, t], F32)           # k-tilde transposed
            if bf:
                qtb = big.tile([dk, t], BF16)
                ktb = big.tile([dk, t], BF16)
            sp = big.tile([dk, t], F32)
            spc = big.tile([dk, t], F32)
            dlast = big.tile([dk, nch], F32)
            vg = big.tile([128, nch, 2 * dv], F32)               # v | g
            sw = big.tile([128, nch, dv], F32)    # g*sigmoid(g)

            spc_v = spc[:].rearrange("p (c l) -> p c l", l=C)

            # scan reset mask: 0 at chunk starts, 1 elsewhere -> one scan per
            # 512-slice does 4 independent per-chunk cumsums
            mres = wt.tile([dk, 512], F32)
            nc.vector.memset(mres[:], 1.0)
            mres_v = mres[:].rearrange("p (c l) -> p c l", l=C)
            nc.vector.memset(mres_v[:, :, 0:1], 0.0)
            ones_sb = wt.tile([dk, 1], F32)
            nc.vector.memset(ones_sb[:], 1.0)

            # ---- x^T load (HWDGE; host supplies transposed x). First slice
            # split into 128-col pieces so the first matmuls start sooner.
            for i in range(4):
                nc.sync.dma_start(xT[:, :, i * C:(i + 1) * C],
                                  xt_d[:, :, i * C:(i + 1) * C])
            for j in range(1, nts):
                nc.sync.dma_start(xT[:, :, j * 512:(j + 1) * 512],
                                  xt_d[:, :, j * 512:(j + 1) * 512])

            def emit_proj(j):
                ts = slice(j * 512, (j + 1) * 512)
                # gate chain: z -> sp = ln(1+exp(-z-b)) -> masked-reset cumsum
                pg = pp.tile([dk, 512], F32, tag="P")
                for d4 in range(4):
                    nc.tensor.matmul(pg[:], wgk_sb[:, d4, :], xT[:, d4, ts],
                                     start=(d4 == 0), stop=(d4 == 3))
                eg = sm.tile([dk, 512], F32, tag="eg")
                nc.scalar.activation(out=eg[:], in_=pg[:], func=AF.Exp,
                                     scale=-1.0, bias=nb_sb[:])
                nc.scalar.activation(out=sp[:, ts], in_=eg[:], func=AF.Ln,
                                     bias=ones_sb[:])
                nc.vector.tensor_tensor_scan(
                    out=spc[:, ts], data0=mres[:], data1=sp[:, ts],
                    initial=0.0, op0=OP.mult, op1=OP.add)
                nc.scalar.activation(
                    out=dlast[:, 4 * j:4 * j + 4],
                    in_=spc_v[:, 4 * j:4 * j + 4, C - 1:C],
                    func=AF.Exp, scale=-1.0 / GATE_NORM)
                # decay factors, stacked [q-rows | k-rows] to match pqk psum
                ee = sm.tile([128, 512], F32, tag="ee")
                nc.scalar.activation(out=ee[0:dk, :], in_=spc[:, ts], func=AF.Exp,
                                     scale=-1.0 / GATE_NORM, bias=lnsc_sb[:])
                nc.scalar.activation(out=ee[dk:2 * dk, :], in_=spc[:, ts],
                                     func=AF.Exp, scale=1.0 / GATE_NORM)

                # q|k projection (fp32r), decay applied on psum eviction
                pqk = pp.tile([128, 512], F32, tag="P")
                for d4 in range(4):
                    nc.tensor.matmul(pqk[:], wqk_sb[:, d4, :], xT[:, d4, ts],
                                     start=(d4 == 0), stop=(d4 == 3))
                nc.vector.tensor_mul(out=qt[:, ts], in0=pqk[0:dk, :],
                                     in1=ee[0:dk, :])
                nc.vector.tensor_mul(out=kt[:, ts], in0=pqk[dk:2 * dk, :],
                                     in1=ee[dk:2 * dk, :])
                if bf:
                    nc.vector.tensor_copy(out=qtb[:, ts], in_=qt[:, ts])
                    nc.vector.tensor_copy(out=ktb[:, ts], in_=kt[:, ts])

                # v|g natural projections
                for i in range(4):
                    tt = 4 * j + i
                    pn = pp.tile([128, 2 * dv], F32, tag="P")
                    for d4 in range(4):
                        nc.tensor.matmul(pn[:],
                                         xT[:, d4, tt * C:(tt + 1) * C],
                                         wvg_sb[:, d4, :],
                                         start=(d4 == 0), stop=(d4 == 3))
                    nc.vector.tensor_copy(out=vg[:, tt, :], in_=pn[:])

                # swish(g) = g * sigmoid(g) = g / (1 + exp(-g))
                gsl = vg[:, 4 * j:4 * j + 4, dv:2 * dv]
                eg2 = sm.tile([128, 4, dv], F32, tag="eg2")
                nc.scalar.activation(out=eg2[:], in_=gsl, func=AF.Exp, scale=-1.0)
                nc.vector.tensor_scalar_add(out=eg2[:], in0=eg2[:], scalar1=1.0)
                sg2 = sm.tile([128, 4, dv], F32, tag="sg2")
                nc.vector.reciprocal_approx_fast(out=sg2[:], in_=eg2[:])
                nc.vector.tensor_mul(out=sw[:, 4 * j:4 * j + 4, :],
                                     in0=sg2[:], in1=gsl)

            for j in range(nts):
                emit_proj(j)

            # ---- chunked recurrence ----
            S_prev = ck.tile([dk, dv], F32, tag="S")
            nc.vector.memset(S_prev[:], 0.0)
            for c in range(nch):
                cs = slice(c * C, (c + 1) * C)
                v_c = vg[:, c, 0:dv]
                kt_c = (ktb if bf else kt)[:, cs]
                qt_c = (qtb if bf else qt)[:, cs]

                pat = pc.tile([C, C], F32, tag="C")
                nc.tensor.matmul(pat[:], kt_c, qt_c, start=True, stop=True)
                atm = ck.tile([C, C], BF16 if bf else F32, tag="atm")
                nc.vector.tensor_mul(out=atm[:], in0=pat[:], in1=um_sb[:])

                pkt = pc.tile([C, dk], BF16 if bf else F32, tag="C")
                nc.tensor.transpose(pkt[:], kt_c,
                                    (idb_sb if bf else id_sb)[0:dk, 0:dk])
                ktn = ck.tile([C, dk], BF16 if bf else F32, tag="ktn")
                nc.scalar.copy(ktn[:], pkt[:])

                po = pc.tile([C, dv], F32, tag="C")
                nc.tensor.matmul(po[:], atm[:], v_c, start=True, stop=False)
                nc.tensor.matmul(po[:], qt[:, cs], S_prev[:],
                                 start=False, stop=True)

                pds = pc.tile([dk, dv], F32, tag="C")
                nc.tensor.matmul(pds[:], ktn[:], v_c, start=True, stop=True)
                S_new = ck.tile([dk, dv], F32, tag="S")
                nc.vector.tensor_add(out=S_new[:], in0=S_prev[:], in1=pds[:])
                nc.vector.tensor_scalar_mul(out=S_new[:], in0=S_new[:],
                                            scalar1=dlast[:, c:c + 1])
                S_prev = S_new

                # rmsnorm + gate
                scr = ck.tile([C, dv], F32, tag="scr")
                ms = ck.tile([C, 1], F32, tag="ms")
                nc.scalar.activation(out=scr[:], in_=po[:], func=AF.Square,
                                     accum_out=ms[:])
                lnv = ck.tile([C, 1], F32, tag="lnv")
                nc.scalar.activation(out=lnv[:], in_=ms[:], func=AF.Ln,
                                     scale=1.0 / dv, bias=eps_sb[:])
                rstd = ck.tile([C, 1], F32, tag="rstd")
                nc.scalar.activation(out=rstd[:], in_=lnv[:], func=AF.Exp,
                                     scale=-0.5)
                on = ck.tile([C, dv], F32, tag="on")
                nc.scalar.mul(on[:], po[:], rstd[:])
                nc.vector.tensor_mul(out=on[:], in0=on[:], in1=sw[:, c, :])

                # transpose + fused output head
                pot = pc.tile([dv, C], F32, tag="C")
                nc.tensor.transpose(pot[:], on[:], id_sb[:])
                ots = ck.tile([dv, C], F32, tag="ots")
                nc.scalar.copy(ots[:], pot[:])
                p10 = pc.tile([C, 10], F32, tag="C")
                nc.tensor.matmul(p10[:], ots[:], wf_sb[:], start=True, stop=True)
                o10 = ck.tile([C, 10], F32, tag="o10")
                nc.vector.tensor_copy(o10[:], p10[:])
                nc.sync.dma_start(out_d[cs, :], o10[:])

    nc.compile()
    return nc


def _prep_inputs(inputs, t=T):
    """Per-core input dicts: core = 4*b + h."""
    ins = {k: np.ascontiguousarray(np.asarray(v, dtype=np.float32))
           for k, v in inputs.items()}
    x, Wq, Wk, Wv, Wg = ins["x"], ins["Wq"], ins["Wk"], ins["Wv"], ins["Wg"]
    Wgk12 = (ins["Wgk1"].astype(np.float64) @ ins["Wgk2"].astype(np.float64))
    bgk2, gnorm = ins["bgk2"], ins["gnorm_w"]
    Wo, Whead = ins["Wo"], ins["Whead"]
    nch = t // C

    um = (np.arange(C)[:, None] <= np.arange(C)[None, :]).astype(np.float32)
    ident = np.eye(128, dtype=np.float32)

    def chunk_w(w):  # [512, n] -> [128, 4, n]
        return np.ascontiguousarray(w.reshape(4, 128, -1).transpose(1, 0, 2))

    in_maps = []
    for core in range(8):
        b, h = divmod(core, 4)
        wf = ((gnorm[:, None].astype(np.float64)
               * Wo[h * dv:(h + 1) * dv, :].astype(np.float64))
              @ Whead.astype(np.float64)).astype(np.float32)
        in_maps.append({
            "xt": np.ascontiguousarray(
                x[b, :t].T.reshape(4, 128, t).transpose(1, 0, 2)),
            "wqk": chunk_w(np.concatenate(
                [Wq[:, h * dk:(h + 1) * dk], Wk[:, h * dk:(h + 1) * dk]], 1)),
            "wvg": chunk_w(np.concatenate(
                [Wv[:, h * dv:(h + 1) * dv], Wg[:, h * dv:(h + 1) * dv]], 1)),
            "wgk12": chunk_w(Wgk12[:, h * dk:(h + 1) * dk].astype(np.float32)),
            "wfused": np.ascontiguousarray(wf),
            "nbgk2": np.ascontiguousarray(-bgk2[h * dk:(h + 1) * dk, None]),
            "umask": um,
            "ident": ident,
        })
    return in_maps


def _gather(results, inputs, t=T):
    bhead = np.asarray(inputs["bhead"], dtype=np.float32)
    out = np.zeros((B, t, 10), np.float32)
    for core in range(8):
        b = core // 4
        out[b] += results[core]["out10"]
    out += bhead[None, None, :]
    return out


def run(inputs, trace=False, **kw):
    from concourse.bass_utils import run_bass_kernel_spmd
    if "nc" not in _CACHE:
        _CACHE["nc"] = build()
    nc = _CACHE["nc"]
    in_maps = _prep_inputs(inputs)
    res = run_bass_kernel_spmd(nc, in_maps, core_ids=list(range(8)),
                               trace=trace, **kw)
    return _gather(res.results, inputs), res


def kernel(**inputs) -> np.ndarray:
    out, _ = run(inputs, trace=False)
    return out



# revision 12
# speedup vs baseline: 1.1074x; 1.1074x over previous
"""Gated linear attention (GLA) Bass kernel for Trainium2, 8 NeuronCores.

Sharding: one core per (batch, head) pair -- B=2 x H=4 = 8 cores.

v2 redesign vs baseline (145us):
  - bf16 throughout: x, weights, chunk matmuls (4x PE rate at 128-col moving
    dim; FWL weight loads; half the HBM traffic for x).
  - transposed-o formulation: po^T[dv,t] = v^T atm + S^T q~ accumulates both
    recurrence terms in one psum group, killing the per-chunk output
    transpose + copy of the baseline.
  - phases pipelined across chunks instead of a serial per-chunk chain:
    B1 (pat/ktn/pds/po-intra, independent per chunk), B2 (state scan
    U_c = U_{c-1}*d_{c-1} + pds_c as one fused vector op reading psum),
    B3 (po += S^T q~), C (rmsnorm via matmul-with-ones + batched Ln/Exp).
  - rmsnorm rstd applied on the 10-col head output instead of the dv-wide
    tensor; sum-of-squares via PE matmul against ones.
  - decay exp computed as ONE [128,512] activation (q|k halves stacked with
    pre-negated sign); softmax scale folded into Wq on host.
  - swish split across scalar/gpsimd/vector; cumsum j>0 on gpsimd.
  - x streamed t-major [128,T,4] so time slices are contiguous DMAs;
    weights packed into 2 DMAs ordered ahead of the x stream.
"""
import sys, os
sys.path.insert(0, "/opt/trn_rl_repo")

import numpy as np

B, T, D = 2, 2048, 512
H = 4
dk, dv = 64, 128          # per-head key/value dims
C = 128                   # chunk length
GATE_NORM = 16.0
EPS = 1e-5
SCALE = dk ** -0.5

_CACHE = {}


def build(t=T):
    import concourse.bass as bass  # noqa: F401
    from concourse import bacc, mybir
    import concourse.tile as tile
    import concourse.hw_specs as hw_specs

    F32 = mybir.dt.float32
    BF16 = mybir.dt.bfloat16
    AF = mybir.ActivationFunctionType
    OP = mybir.AluOpType

    # Keep every activation func we use in one table (see baseline comment):
    # Exp, Ln, Square, Copy, Identity all live in natural_log_exp_and_others.
    need = {AF.Exp, AF.Ln, AF.Square, AF.Copy, AF.Identity}
    keep = "natural_log_exp_and_others"
    tabs = hw_specs.get_activation_tables("gen3")
    if keep in tabs and need <= tabs[keep]:
        for name, s in tabs.items():
            if name != keep:
                s -= need

    nch = t // C              # chunks
    nts = t // 512            # 512-wide time slices
    assert t % 512 == 0 and nch % 4 == 0
    ngrp = nch // 4

    nc = bacc.Bacc("TRN2", target_bir_lowering=False, debug=False)

    xt_d = nc.dram_tensor("xt", [128, t, 4], BF16, kind="ExternalInput")
    w1_d = nc.dram_tensor("w1", [128, 4, dk], BF16, kind="ExternalInput")
    w2_d = nc.dram_tensor("w2", [128, 4, 2 * C + dv], BF16, kind="ExternalInput")
    um_d = nc.dram_tensor("umask", [C, C], F32, kind="ExternalInput")
    id_d = nc.dram_tensor("ident", [dk, dk], BF16, kind="ExternalInput")
    wf_d = nc.dram_tensor("wfused", [dv, 10], BF16, kind="ExternalInput")
    nb_d = nc.dram_tensor("nbgk2", [dk, 1], F32, kind="ExternalInput")
    out_d = nc.dram_tensor("out10", [128, nch, 10], F32, kind="ExternalOutput")

    with tile.TileContext(nc) as tc:
        with (
            tc.tile_pool(name="wt", bufs=1) as wt,
            tc.tile_pool(name="sm", bufs=2) as sm,
            tc.tile_pool(name="ck", bufs=3) as ck,
            tc.tile_pool(name="pp", bufs=4, space="PSUM") as pp,
            tc.tile_pool(name="pq", bufs=1, space="PSUM") as pq,
        ):
            # ---- persistent SBUF ----
            xt = wt.tile([128, t, 4], BF16)
            w1 = wt.tile([128, 4, dk], BF16)           # wgk (low-rank fused)
            w2 = wt.tile([128, 4, 2 * C + dv], BF16)   # [wqk 128 | wg 128 | wv 128]
            um_sb = wt.tile([C, C], F32)
            id_sb = wt.tile([dk, dk], BF16)
            wf_sb = wt.tile([dv, 10], BF16)
            nb_sb = wt.tile([dk, 1], F32)

            # input DMAs: weights ahead of the x stream on the sync queue
            nc.sync.dma_start(w1[:], w1_d[:])
            nc.sync.dma_start(xt[:, 0:512, :], xt_d[:, 0:512, :])
            nc.sync.dma_start(w2[:], w2_d[:])
            for j in range(1, nts):
                nc.sync.dma_start(xt[:, j * 512:(j + 1) * 512, :],
                                  xt_d[:, j * 512:(j + 1) * 512, :])
            nc.gpsimd.dma_start(um_sb[:], um_d[:])
            nc.gpsimd.dma_start(id_sb[:], id_d[:])
            nc.gpsimd.dma_start(wf_sb[:], wf_d[:])
            nc.gpsimd.dma_start(nb_sb[:], nb_d[:])

            wqk = w2[:, :, 0:C]
            wg = w2[:, :, C:2 * C]
            wv = w2[:, :, 2 * C:2 * C + dv]

            ones64 = wt.tile([dk, 1], F32)
            nc.vector.memset(ones64[:], 1.0)
            onesbf = wt.tile([128, 1], BF16)
            nc.vector.memset(onesbf[:], 1.0)
            eps_sb = wt.tile([128, 1], F32)
            nc.vector.memset(eps_sb[:], EPS)
            # scan reset mask: 0 at chunk starts
            mres = wt.tile([dk, 512], F32)
            nc.vector.memset(mres[:], 1.0)
            mres_v = mres[:].rearrange("p (c l) -> p c l", l=C)
            nc.vector.memset(mres_v[:, :, 0:1], 0.0)

            # big SBUF activations
            spc2 = wt.tile([128, t], F32)     # rows 0:64 = -cumsum, 64:128 = +cumsum
            qt_t = wt.tile([dk, t], BF16)     # q~^T
            kt_t = wt.tile([dk, t], BF16)     # k~^T
            swt = wt.tile([dv, t], BF16)      # swish(g)^T
            vsb = wt.tile([128, nch, dv], BF16)
            dlast = wt.tile([dk, nch], F32)
            ktn = wt.tile([C, nch, dk], BF16)
            Sb = wt.tile([dk, nch, dv], BF16)
            obuf = wt.tile([128, nch, 10], F32)

            spc2_k = spc2[64:128, :]
            spc2_q = spc2[0:64, :]
            dl_src = spc2_k.rearrange("p (c l) -> p c l", l=C)

            # ---- PSUM (bank-granular: 4 + 1 + 1 + 1 = 7 of 8 banks) ----
            prs = pq.tile([128, 512], F32)    # pat 2 slots | pds 2 | pkt 2
            pvt = pq.tile([128, 4, dv], F32)  # v projection, 4 rotating slots
            pot = pq.tile([128, 4, C], F32)   # po^T, 4 rotating slots
            # ssq cols | p10 cols | 2 pkt slots (bf16-bitcast, 32 f32 cols ea)
            pt2 = pq.tile([128, nch + nch * 10 + dk], F32)

            def pat_s(c):
                return prs[:, (c % 2) * C:(c % 2) * C + C]

            def pds_s(c):
                return prs[0:dk, 256 + (c % 2) * C:256 + (c % 2) * C + C]

            def pkt_s(c):
                base = nch + nch * 10 + (c % 2) * (dk // 2)
                return pt2[:, base:base + dk // 2].bitcast(BF16)

            def ssq_s(c):
                return pt2[:, c:c + 1]

            def p10_s(c):
                return pt2[:, nch + c * 10:nch + (c + 1) * 10]

            # ---------------- phase A: projections ----------------
            for j in range(nts):
                ts = slice(j * 512, (j + 1) * 512)
                xs = xt[:, ts, :]

                # gate chain
                pz = pp.tile([dk, 512], F32, tag="P")
                for d4 in range(4):
                    nc.tensor.matmul(pz[:], w1[:, d4, :], xs[:, :, d4],
                                     start=(d4 == 0), stop=(d4 == 3))
                eg = sm.tile([dk, 512], BF16, tag="eg")
                nc.scalar.activation(out=eg[:], in_=pz[:], func=AF.Exp,
                                     scale=-1.0, bias=nb_sb[:])
                sp = sm.tile([dk, 512], F32, tag="sp")
                nc.scalar.activation(out=sp[:], in_=eg[:], func=AF.Ln,
                                     bias=ones64[:])
                nc.vector.tensor_tensor_scan(
                    out=spc2_k[:, ts], data0=mres[:], data1=sp[:],
                    initial=0.0, op0=OP.mult, op1=OP.add)
                nc.gpsimd.tensor_scalar_mul(out=spc2_q[:, ts],
                                            in0=spc2_k[:, ts], scalar1=-1.0)
                nc.scalar.activation(out=dlast[:, 4 * j:4 * j + 4],
                                     in_=dl_src[:, 4 * j:4 * j + 4, C - 1:C],
                                     func=AF.Exp, scale=-1.0 / GATE_NORM)
                ee = sm.tile([128, 512], F32, tag="ee")
                nc.scalar.activation(out=ee[:], in_=spc2[:, ts], func=AF.Exp,
                                     scale=1.0 / GATE_NORM)

                # q|k projection + decay
                pqk = pp.tile([128, 512], F32, tag="P")
                for d4 in range(4):
                    nc.tensor.matmul(pqk[:], wqk[:, d4, :], xs[:, :, d4],
                                     start=(d4 == 0), stop=(d4 == 3))
                nc.vector.tensor_tensor(out=qt_t[:, ts], in0=pqk[0:dk, :],
                                        in1=ee[0:dk, :], op=OP.mult)
                nc.vector.tensor_tensor(out=kt_t[:, ts], in0=pqk[64:128, :],
                                        in1=ee[64:128, :], op=OP.mult)

                # g^T projection + swish
                pgt = pp.tile([128, 512], F32, tag="P")
                for d4 in range(4):
                    nc.tensor.matmul(pgt[:], wg[:, d4, :], xs[:, :, d4],
                                     start=(d4 == 0), stop=(d4 == 3))
                eg2 = sm.tile([dv, 512], BF16, tag="eg2")
                nc.scalar.activation(out=eg2[:], in_=pgt[:], func=AF.Exp,
                                     scale=-1.0)
                s1 = sm.tile([dv, 512], F32, tag="s1")
                nc.gpsimd.tensor_scalar_add(out=s1[:], in0=eg2[:], scalar1=1.0)
                s2 = sm.tile([dv, 512], F32, tag="s2")
                nc.vector.reciprocal_approx_fast(out=s2[:], in_=s1[:])
                nc.vector.tensor_tensor(out=swt[:, ts], in0=pgt[:], in1=s2[:],
                                        op=OP.mult)

                # v natural projections
                for i in range(4):
                    tt = 4 * j + i
                    pvs = pvt[:, tt % 4, :]
                    for d4 in range(4):
                        nc.tensor.matmul(pvs, xs[:, i * C:(i + 1) * C, d4],
                                         wv[:, d4, :],
                                         start=(d4 == 0), stop=(d4 == 3))
                    if i % 2 == 0:
                        nc.scalar.copy(vsb[:, tt, :], pvs)
                    else:
                        nc.vector.tensor_copy(vsb[:, tt, :], pvs)

            # ---------------- phase B/C: chunked recurrence ----------------
            U = [None, None]

            def emit_c(c):
                """post-processing of chunk c (po complete)."""
                po = pot[:, c % 4, :]
                ot = ck.tile([dv, C], BF16, tag="ot")
                nc.vector.tensor_tensor(out=ot[:], in0=po, in1=swt[:, sl(c)],
                                        op=OP.mult)
                sq = ck.tile([dv, C], BF16, tag="sq")
                nc.scalar.activation(out=sq[:], in_=po, func=AF.Square)
                nc.tensor.matmul(ssq_s(c), sq[:], onesbf[:],
                                 start=True, stop=True)
                nc.tensor.matmul(p10_s(c), ot[:], wf_sb[:],
                                 start=True, stop=True)
                if c % 4 == 3:
                    g = c // 4
                    lnv = ck.tile([128, 4], F32, tag="lnv")
                    nc.scalar.activation(out=lnv[:], in_=pt2[:, 4 * g:4 * g + 4],
                                         func=AF.Ln, scale=1.0 / dv,
                                         bias=eps_sb[:])
                    rstd = ck.tile([128, 4], F32, tag="rstd")
                    nc.scalar.activation(out=rstd[:], in_=lnv[:], func=AF.Exp,
                                         scale=-0.5)
                    p10g = pt2[:, nch + g * 40:nch + (g + 1) * 40]
                    nc.vector.tensor_tensor(
                        out=obuf[:, 4 * g:4 * g + 4, :],
                        in0=p10g.rearrange("p (c n) -> p c n", n=10),
                        in1=rstd[:].unsqueeze(2).broadcast_to([128, 4, 10]),
                        op=OP.mult)
                    if nch >= 8 and g == ngrp // 2 - 1:
                        nc.sync.dma_start(out_d[:, 0:nch // 2, :],
                                          obuf[:, 0:nch // 2, :])
                    elif g == ngrp - 1:
                        lo = nch // 2 if nch >= 8 else 0
                        nc.sync.dma_start(out_d[:, lo:nch, :],
                                          obuf[:, lo:nch, :])

            def sl(c):
                return slice(c * C, (c + 1) * C)

            for c in range(nch):
                cs = sl(c)
                qt_c = qt_t[:, cs]
                kt_c = kt_t[:, cs]
                v_c = vsb[:, c, :]
                # B1
                pat = pat_s(c)
                nc.tensor.matmul(pat, kt_c, qt_c, start=True, stop=True)
                atm = ck.tile([C, C], BF16, tag="atm")
                nc.vector.tensor_tensor(out=atm[:], in0=pat, in1=um_sb[:],
                                        op=OP.mult)
                pkt = pkt_s(c)
                nc.tensor.transpose(pkt, kt_c, id_sb[:])
                nc.scalar.copy(ktn[:, c, :], pkt)
                pds = pds_s(c)
                nc.tensor.matmul(pds, ktn[:, c, :], v_c, start=True, stop=True)
                po = pot[:, c % 4, :]
                nc.tensor.matmul(po, v_c, atm[:], start=True, stop=(c == 0))
                # B2: U_c = U_{c-1} * d_{c-1} + pds_c ; Sb_c = bf16(U_c * d_c)
                Uc = ck.tile([dk, dv], F32, tag="U")
                if c == 0:
                    nc.vector.tensor_copy(Uc[:], pds)
                else:
                    nc.vector.scalar_tensor_tensor(
                        out=Uc[:], in0=U[(c - 1) % 2][:],
                        scalar=dlast[:, c - 1:c], op0=OP.mult,
                        in1=pds, op1=OP.add)
                U[c % 2] = Uc
                nc.gpsimd.tensor_tensor(
                    out=Sb[:, c, :], in0=Uc[:],
                    in1=dlast[:, c:c + 1].broadcast_to([dk, dv]), op=OP.mult)
                # B3: po_c += S_{c-1}^T q~_c
                if c > 0:
                    nc.tensor.matmul(po, Sb[:, c - 1, :], qt_c,
                                     start=False, stop=True)
                # C for previous chunk (1-chunk software pipeline)
                if c > 0:
                    emit_c(c - 1)
            emit_c(nch - 1)

    nc.compile()
    return nc


def _prep_inputs(inputs, t=T):
    """Per-core input dicts: core = 4*b + h."""
    import ml_dtypes
    bf16 = ml_dtypes.bfloat16
    ins = {k: np.ascontiguousarray(np.asarray(v, dtype=np.float32))
           for k, v in inputs.items()}
    x, Wq, Wk, Wv, Wg = ins["x"], ins["Wq"], ins["Wk"], ins["Wv"], ins["Wg"]
    Wgk12 = (ins["Wgk1"].astype(np.float64) @ ins["Wgk2"].astype(np.float64))
    bgk2, gnorm = ins["bgk2"], ins["gnorm_w"]
    Wo, Whead = ins["Wo"], ins["Whead"]

    um = (np.arange(C)[:, None] <= np.arange(C)[None, :]).astype(np.float32)
    ident = np.eye(dk, dtype=np.float32).astype(bf16)

    def chunk_w(w):  # [512, n] -> [128, 4, n]
        return np.ascontiguousarray(
            w.reshape(4, 128, -1).transpose(1, 0, 2).astype(bf16))

    in_maps = []
    for core in range(8):
        b, h = divmod(core, 4)
        wf = ((gnorm[:, None].astype(np.float64)
               * Wo[h * dv:(h + 1) * dv, :].astype(np.float64))
              @ Whead.astype(np.float64)).astype(np.float32)
        w2 = np.concatenate(
            [Wq[:, h * dk:(h + 1) * dk] * SCALE, Wk[:, h * dk:(h + 1) * dk],
             Wg[:, h * dv:(h + 1) * dv], Wv[:, h * dv:(h + 1) * dv]], axis=1)
        in_maps.append({
            "xt": np.ascontiguousarray(
                x[b, :t].reshape(t, 4, 128).transpose(2, 0, 1).astype(bf16)),
            "w1": chunk_w(Wgk12[:, h * dk:(h + 1) * dk].astype(np.float32)),
            "w2": chunk_w(w2),
            "umask": um,
            "ident": np.ascontiguousarray(ident),
            "wfused": np.ascontiguousarray(wf.astype(bf16)),
            "nbgk2": np.ascontiguousarray(-bgk2[h * dk:(h + 1) * dk, None]),
        })
    return in_maps


def _gather(results, inputs, t=T):
    bhead = np.asarray(inputs["bhead"], dtype=np.float32)
    out = np.zeros((B, t, 10), np.float32)
    for core in range(8):
        b = core // 4
        r = results[core]["out10"]          # [128, nch, 10]
        out[b] += r.transpose(1, 0, 2).reshape(t, 10)
    out += bhead[None, None, :]
    return out


def run(inputs, trace=False, **kw):
    from concourse.bass_utils import run_bass_kernel_spmd
    if "nc" not in _CACHE:
        _CACHE["nc"] = build()
    nc = _CACHE["nc"]
    in_maps = _prep_inputs(inputs)
    res = run_bass_kernel_spmd(nc, in_maps, core_ids=list(range(8)),
                               trace=trace, **kw)
    return _gather(res.results, inputs), res


def kernel(**inputs) -> np.ndarray:
    out, _ = run(inputs, trace=False)
    return out


# revision 17
# speedup vs baseline: 1.7439x; 1.5748x over previous
"""Gated linear attention (GLA) Bass kernel for Trainium2, 8 NeuronCores.

Sharding: one core per (batch, head) pair -- B=2 x H=4 = 8 cores.

v2 redesign vs baseline (145us):
  - bf16 throughout: x, weights, chunk matmuls (4x PE rate at 128-col moving
    dim; FWL weight loads; half the HBM traffic for x).
  - transposed-o formulation: po^T[dv,t] = v^T atm + S^T q~ accumulates both
    recurrence terms in one psum group, killing the per-chunk output
    transpose + copy of the baseline.
  - phases pipelined across chunks instead of a serial per-chunk chain:
    B1 (pat/ktn/pds/po-intra, independent per chunk), B2 (state scan
    U_c = U_{c-1}*d_{c-1} + pds_c as one fused vector op reading psum),
    B3 (po += S^T q~), C (rmsnorm via matmul-with-ones + batched Ln/Exp).
  - rmsnorm rstd applied on the 10-col head output instead of the dv-wide
    tensor; sum-of-squares via PE matmul against ones.
  - decay exp computed as ONE [128,512] activation (q|k halves stacked with
    pre-negated sign); softmax scale folded into Wq on host.
  - swish split across scalar/gpsimd/vector; cumsum j>0 on gpsimd.
  - x streamed t-major [128,T,4] so time slices are contiguous DMAs;
    weights packed into 2 DMAs ordered ahead of the x stream.
"""
import sys, os
sys.path.insert(0, "/opt/trn_rl_repo")

import numpy as np

B, T, D = 2, 2048, 512
H = 4
dk, dv = 64, 128          # per-head key/value dims
C = 128                   # chunk length
GATE_NORM = 16.0
EPS = 1e-5
SCALE = dk ** -0.5

_CACHE = {}


def build(t=T):
    import concourse.bass as bass  # noqa: F401
    from concourse import bacc, mybir
    import concourse.tile as tile
    import concourse.hw_specs as hw_specs

    F32 = mybir.dt.float32
    BF16 = mybir.dt.bfloat16
    AF = mybir.ActivationFunctionType
    OP = mybir.AluOpType

    # Keep every activation func we use in one table (see baseline comment):
    # Exp, Ln, Square, Copy, Identity all live in natural_log_exp_and_others.
    need = {AF.Exp, AF.Ln, AF.Square, AF.Copy, AF.Identity}
    keep = "natural_log_exp_and_others"
    tabs = hw_specs.get_activation_tables("gen3")
    if keep in tabs and need <= tabs[keep]:
        for name, s in tabs.items():
            if name != keep:
                s -= need

    nch = t // C              # chunks
    nts = t // 512            # 512-wide time slices
    assert t % 512 == 0 and nch % 4 == 0
    ngrp = nch // 4

    nc = bacc.Bacc("TRN2", target_bir_lowering=False, debug=False)

    xt_d = nc.dram_tensor("xt", [128, t, 4], BF16, kind="ExternalInput")
    w1_d = nc.dram_tensor("w1", [128, 4, dk], BF16, kind="ExternalInput")
    w2_d = nc.dram_tensor("w2", [128, 4, 2 * C + dv], BF16, kind="ExternalInput")
    um_d = nc.dram_tensor("umask", [C, C], F32, kind="ExternalInput")
    id_d = nc.dram_tensor("ident", [dk, dk], BF16, kind="ExternalInput")
    wf_d = nc.dram_tensor("wfused", [dv, 10], BF16, kind="ExternalInput")
    nb_d = nc.dram_tensor("nbgk2", [dk, 1], F32, kind="ExternalInput")
    out_d = nc.dram_tensor("out10", [128, nch, 10], F32, kind="ExternalOutput")

    with tile.TileContext(nc) as tc:
        with (
            tc.tile_pool(name="wt", bufs=1) as wt,
            tc.tile_pool(name="sm", bufs=2) as sm,
            tc.tile_pool(name="ck", bufs=3) as ck,
            tc.tile_pool(name="pp", bufs=4, space="PSUM") as pp,
            tc.tile_pool(name="pq", bufs=1, space="PSUM") as pq,
        ):
            # ---- persistent SBUF ----
            xt = wt.tile([128, t, 4], BF16)
            w1 = wt.tile([128, 4, dk], BF16)           # wgk (low-rank fused)
            w2 = wt.tile([128, 4, 2 * C + dv], BF16)   # [wqk 128 | wg 128 | wv 128]
            um_sb = wt.tile([C, C], F32)
            id_sb = wt.tile([dk, dk], BF16)
            wf_sb = wt.tile([dv, 10], BF16)
            nb_sb = wt.tile([dk, 1], F32)

            # input DMAs: weights ahead of the x stream on the sync queue
            nc.sync.dma_start(w1[:], w1_d[:])
            nc.sync.dma_start(xt[:, 0:512, :], xt_d[:, 0:512, :])
            nc.sync.dma_start(w2[:], w2_d[:])
            for j in range(1, nts):
                nc.sync.dma_start(xt[:, j * 512:(j + 1) * 512, :],
                                  xt_d[:, j * 512:(j + 1) * 512, :])
            nc.gpsimd.dma_start(um_sb[:], um_d[:])
            nc.gpsimd.dma_start(id_sb[:], id_d[:])
            nc.gpsimd.dma_start(wf_sb[:], wf_d[:])
            nc.gpsimd.dma_start(nb_sb[:], nb_d[:])

            wqk = w2[:, :, 0:C]
            wg = w2[:, :, C:2 * C]
            wv = w2[:, :, 2 * C:2 * C + dv]

            ones64 = wt.tile([dk, 1], F32)
            nc.vector.memset(ones64[:], 1.0)
            onesbf = wt.tile([128, 1], BF16)
            nc.vector.memset(onesbf[:], 1.0)
            eps_sb = wt.tile([128, 1], F32)
            nc.vector.memset(eps_sb[:], EPS)
            # scan reset mask: 0 at chunk starts
            mres = wt.tile([dk, 512], F32)
            nc.vector.memset(mres[:], 1.0)
            mres_v = mres[:].rearrange("p (c l) -> p c l", l=C)
            nc.vector.memset(mres_v[:, :, 0:1], 0.0)

            # big SBUF activations
            spc = wt.tile([dk, t], F32)       # per-chunk cumsum of softplus
            qt_t = wt.tile([dk, t], BF16)     # q~^T
            kt_t = wt.tile([dk, t], BF16)     # k~^T
            swt = wt.tile([dv, t], BF16)      # swish(g)^T
            vsb = wt.tile([128, nch, dv], BF16)
            dlast = wt.tile([dk, nch], F32)
            ktn = wt.tile([C, nch, dk], BF16)
            Sb = wt.tile([dk, nch, dv], BF16)
            obuf = wt.tile([128, nch, 10], F32)

            dl_src = spc[:].rearrange("p (c l) -> p c l", l=C)

            # ---- PSUM (bank-granular: 4 + 1 + 1 + 1 = 7 of 8 banks) ----
            prs = pq.tile([128, 512], F32)    # pat 2 slots | pds 2 | pkt 2
            pvt = pq.tile([128, 4, dv], F32)  # v projection, 4 rotating slots
            pot = pq.tile([128, 4, C], F32)   # po^T, 4 rotating slots
            # ssq cols | p10 cols | 2 pkt slots (bf16-bitcast, 32 f32 cols ea)
            pt2 = pq.tile([128, nch + nch * 10 + dk], F32)

            def pat_s(c):
                return prs[:, (c % 2) * C:(c % 2) * C + C]

            def pds_s(c):
                return prs[0:dk, 256 + (c % 2) * C:256 + (c % 2) * C + C]

            def pkt_s(c):
                base = nch + nch * 10 + (c % 2) * (dk // 2)
                return pt2[:, base:base + dk // 2].bitcast(BF16)

            def ssq_s(c):
                return pt2[:, c:c + 1]

            def p10_s(c):
                return pt2[:, nch + c * 10:nch + (c + 1) * 10]

            # ---------------- phase A: projections ----------------
            for j in range(nts):
                ts = slice(j * 512, (j + 1) * 512)
                xs = xt[:, ts, :]

                # gate chain
                pz = pp.tile([dk, 512], F32, tag="P")
                for d4 in range(4):
                    nc.tensor.matmul(pz[:], w1[:, d4, :], xs[:, :, d4],
                                     start=(d4 == 0), stop=(d4 == 3))
                eg = sm.tile([dk, 512], BF16, tag="eg")
                nc.scalar.activation(out=eg[:], in_=pz[:], func=AF.Exp,
                                     scale=-1.0, bias=nb_sb[:])
                sp = sm.tile([dk, 512], F32, tag="sp")
                nc.scalar.activation(out=sp[:], in_=eg[:], func=AF.Ln,
                                     bias=ones64[:])
                nc.vector.tensor_tensor_scan(
                    out=spc[:, ts], data0=mres[:], data1=sp[:],
                    initial=0.0, op0=OP.mult, op1=OP.add)
                nc.scalar.activation(out=dlast[:, 4 * j:4 * j + 4],
                                     in_=dl_src[:, 4 * j:4 * j + 4, C - 1:C],
                                     func=AF.Exp, scale=-1.0 / GATE_NORM)
                eeq = sm.tile([dk, 512], F32, tag="eeq")
                nc.scalar.activation(out=eeq[:], in_=spc[:, ts], func=AF.Exp,
                                     scale=-1.0 / GATE_NORM)
                eek = sm.tile([dk, 512], F32, tag="eek")
                nc.scalar.activation(out=eek[:], in_=spc[:, ts], func=AF.Exp,
                                     scale=1.0 / GATE_NORM)

                # q|k projection + decay
                pqk = pp.tile([128, 512], F32, tag="P")
                for d4 in range(4):
                    nc.tensor.matmul(pqk[:], wqk[:, d4, :], xs[:, :, d4],
                                     start=(d4 == 0), stop=(d4 == 3))
                nc.vector.tensor_tensor(out=qt_t[:, ts], in0=pqk[0:dk, :],
                                        in1=eeq[:], op=OP.mult)
                nc.vector.tensor_tensor(out=kt_t[:, ts], in0=pqk[64:128, :],
                                        in1=eek[:], op=OP.mult)

                # g^T projection + swish
                pgt = pp.tile([128, 512], F32, tag="P")
                for d4 in range(4):
                    nc.tensor.matmul(pgt[:], wg[:, d4, :], xs[:, :, d4],
                                     start=(d4 == 0), stop=(d4 == 3))
                eg2 = sm.tile([dv, 512], BF16, tag="eg2")
                nc.scalar.activation(out=eg2[:], in_=pgt[:], func=AF.Exp,
                                     scale=-1.0)
                s1 = sm.tile([dv, 512], F32, tag="s1")
                nc.vector.tensor_scalar_add(out=s1[:], in0=eg2[:], scalar1=1.0)
                s2 = sm.tile([dv, 512], F32, tag="s2")
                nc.vector.reciprocal_approx_fast(out=s2[:], in_=s1[:])
                nc.vector.tensor_tensor(out=swt[:, ts], in0=pgt[:], in1=s2[:],
                                        op=OP.mult)

                # v natural projections
                for i in range(4):
                    tt = 4 * j + i
                    pvs = pvt[:, tt % 4, :]
                    for d4 in range(4):
                        nc.tensor.matmul(pvs, xs[:, i * C:(i + 1) * C, d4],
                                         wv[:, d4, :],
                                         start=(d4 == 0), stop=(d4 == 3))
                    if i % 2 == 0:
                        nc.scalar.copy(vsb[:, tt, :], pvs)
                    else:
                        nc.vector.tensor_copy(vsb[:, tt, :], pvs)

            # ---------------- phase B/C: chunked recurrence ----------------
            U = [None, None]

            def emit_c(c):
                """post-processing of chunk c (po complete)."""
                po = pot[:, c % 4, :]
                ot = ck.tile([dv, C], BF16, tag="ot")
                nc.vector.tensor_tensor(out=ot[:], in0=po, in1=swt[:, sl(c)],
                                        op=OP.mult)
                sq = ck.tile([dv, C], BF16, tag="sq")
                nc.scalar.activation(out=sq[:], in_=po, func=AF.Square)
                nc.tensor.matmul(ssq_s(c), sq[:], onesbf[:],
                                 start=True, stop=True)
                nc.tensor.matmul(p10_s(c), ot[:], wf_sb[:],
                                 start=True, stop=True)
                if c % 4 == 3:
                    g = c // 4
                    lnv = ck.tile([128, 4], F32, tag="lnv")
                    nc.scalar.activation(out=lnv[:], in_=pt2[:, 4 * g:4 * g + 4],
                                         func=AF.Ln, scale=1.0 / dv,
                                         bias=eps_sb[:])
                    rstd = ck.tile([128, 4], F32, tag="rstd")
                    nc.scalar.activation(out=rstd[:], in_=lnv[:], func=AF.Exp,
                                         scale=-0.5)
                    p10g = pt2[:, nch + g * 40:nch + (g + 1) * 40]
                    nc.vector.tensor_tensor(
                        out=obuf[:, 4 * g:4 * g + 4, :],
                        in0=p10g.rearrange("p (c n) -> p c n", n=10),
                        in1=rstd[:].unsqueeze(2).broadcast_to([128, 4, 10]),
                        op=OP.mult)
                    if nch >= 8 and g == ngrp // 2 - 1:
                        nc.sync.dma_start(out_d[:, 0:nch // 2, :],
                                          obuf[:, 0:nch // 2, :])
                    elif g == ngrp - 1:
                        lo = nch // 2 if nch >= 8 else 0
                        nc.sync.dma_start(out_d[:, lo:nch, :],
                                          obuf[:, lo:nch, :])

            def sl(c):
                return slice(c * C, (c + 1) * C)

            for c in range(nch):
                cs = sl(c)
                qt_c = qt_t[:, cs]
                kt_c = kt_t[:, cs]
                v_c = vsb[:, c, :]
                # B1
                pat = pat_s(c)
                nc.tensor.matmul(pat, kt_c, qt_c, start=True, stop=True)
                atm = ck.tile([C, C], BF16, tag="atm")
                nc.vector.tensor_tensor(out=atm[:], in0=pat, in1=um_sb[:],
                                        op=OP.mult)
                pkt = pkt_s(c)
                nc.tensor.transpose(pkt, kt_c, id_sb[:])
                nc.scalar.copy(ktn[:, c, :], pkt)
                pds = pds_s(c)
                nc.tensor.matmul(pds, ktn[:, c, :], v_c, start=True, stop=True)
                po = pot[:, c % 4, :]
                nc.tensor.matmul(po, v_c, atm[:], start=True, stop=(c == 0))
                # B2: U_c = U_{c-1} * d_{c-1} + pds_c ; Sb_c = bf16(U_c * d_c)
                Uc = ck.tile([dk, dv], F32, tag="U")
                if c == 0:
                    nc.vector.tensor_copy(Uc[:], pds)
                else:
                    nc.vector.scalar_tensor_tensor(
                        out=Uc[:], in0=U[(c - 1) % 2][:],
                        scalar=dlast[:, c - 1:c], op0=OP.mult,
                        in1=pds, op1=OP.add)
                U[c % 2] = Uc
                nc.gpsimd.tensor_tensor(
                    out=Sb[:, c, :], in0=Uc[:],
                    in1=dlast[:, c:c + 1].broadcast_to([dk, dv]), op=OP.mult)
                # B3: po_c += S_{c-1}^T q~_c
                if c > 0:
                    nc.tensor.matmul(po, Sb[:, c - 1, :], qt_c,
                                     start=False, stop=True)
                # C for previous chunk (1-chunk software pipeline)
                if c > 0:
                    emit_c(c - 1)
            emit_c(nch - 1)

    nc.compile()
    return nc


def _prep_inputs(inputs, t=T):
    """Per-core input dicts: core = 4*b + h."""
    import ml_dtypes
    bf16 = ml_dtypes.bfloat16
    ins = {k: np.ascontiguousarray(np.asarray(v, dtype=np.float32))
           for k, v in inputs.items()}
    x, Wq, Wk, Wv, Wg = ins["x"], ins["Wq"], ins["Wk"], ins["Wv"], ins["Wg"]
    Wgk12 = (ins["Wgk1"].astype(np.float64) @ ins["Wgk2"].astype(np.float64))
    bgk2, gnorm = ins["bgk2"], ins["gnorm_w"]
    Wo, Whead = ins["Wo"], ins["Whead"]

    um = (np.arange(C)[:, None] <= np.arange(C)[None, :]).astype(np.float32)
    ident = np.eye(dk, dtype=np.float32).astype(bf16)

    def chunk_w(w):  # [512, n] -> [128, 4, n]
        return np.ascontiguousarray(
            w.reshape(4, 128, -1).transpose(1, 0, 2).astype(bf16))

    in_maps = []
    for core in range(8):
        b, h = divmod(core, 4)
        wf = ((gnorm[:, None].astype(np.float64)
               * Wo[h * dv:(h + 1) * dv, :].astype(np.float64))
              @ Whead.astype(np.float64)).astype(np.float32)
        w2 = np.concatenate(
            [Wq[:, h * dk:(h + 1) * dk] * SCALE, Wk[:, h * dk:(h + 1) * dk],
             Wg[:, h * dv:(h + 1) * dv], Wv[:, h * dv:(h + 1) * dv]], axis=1)
        in_maps.append({
            "xt": np.ascontiguousarray(
                x[b, :t].reshape(t, 4, 128).transpose(2, 0, 1).astype(bf16)),
            "w1": chunk_w(Wgk12[:, h * dk:(h + 1) * dk].astype(np.float32)),
            "w2": chunk_w(w2),
            "umask": um,
            "ident": np.ascontiguousarray(ident),
            "wfused": np.ascontiguousarray(wf.astype(bf16)),
            "nbgk2": np.ascontiguousarray(-bgk2[h * dk:(h + 1) * dk, None]),
        })
    return in_maps


def _gather(results, inputs, t=T):
    bhead = np.asarray(inputs["bhead"], dtype=np.float32)
    out = np.zeros((B, t, 10), np.float32)
    for core in range(8):
        b = core // 4
        r = results[core]["out10"]          # [128, nch, 10]
        out[b] += r.transpose(1, 0, 2).reshape(t, 10)
    out += bhead[None, None, :]
    return out


def run(inputs, trace=False, **kw):
    from concourse.bass_utils import run_bass_kernel_spmd
    if "nc" not in _CACHE:
        _CACHE["nc"] = build()
    nc = _CACHE["nc"]
    in_maps = _prep_inputs(inputs)
    res = run_bass_kernel_spmd(nc, in_maps, core_ids=list(range(8)),
                               trace=trace, **kw)
    return _gather(res.results, inputs), res


def kernel(**inputs) -> np.ndarray:
    out, _ = run(inputs, trace=False)
    return out


# revision 31
# speedup vs baseline: 1.9638x; 1.1261x over previous
"""Gated linear attention (GLA) Bass kernel for Trainium2, 8 NeuronCores.

Sharding: one core per (batch, head) pair -- B=2 x H=4 = 8 cores.

v2 redesign vs baseline (145us):
  - bf16 throughout: x, weights, chunk matmuls (4x PE rate at 128-col moving
    dim; FWL weight loads; half the HBM traffic for x).
  - transposed-o formulation: po^T[dv,t] = v^T atm + S^T q~ accumulates both
    recurrence terms in one psum group, killing the per-chunk output
    transpose + copy of the baseline.
  - phases pipelined across chunks instead of a serial per-chunk chain:
    B1 (pat/ktn/pds/po-intra, independent per chunk), B2 (state scan
    U_c = U_{c-1}*d_{c-1} + pds_c as one fused vector op reading psum),
    B3 (po += S^T q~), C (rmsnorm via matmul-with-ones + batched Ln/Exp).
  - rmsnorm rstd applied on the 10-col head output instead of the dv-wide
    tensor; sum-of-squares via PE matmul against ones.
  - decay exp computed as ONE [128,512] activation (q|k halves stacked with
    pre-negated sign); softmax scale folded into Wq on host.
  - swish split across scalar/gpsimd/vector; cumsum j>0 on gpsimd.
  - x streamed t-major [128,T,4] so time slices are contiguous DMAs;
    weights packed into 2 DMAs ordered ahead of the x stream.
"""
import sys, os
sys.path.insert(0, "/opt/trn_rl_repo")

import numpy as np

B, T, D = 2, 2048, 512
H = 4
dk, dv = 64, 128          # per-head key/value dims
C = 128                   # chunk length
GATE_NORM = 16.0
EPS = 1e-5
SCALE = dk ** -0.5

_CACHE = {}


def build(t=T):
    import concourse.bass as bass  # noqa: F401
    from concourse import bacc, mybir
    import concourse.tile as tile
    import concourse.hw_specs as hw_specs

    F32 = mybir.dt.float32
    BF16 = mybir.dt.bfloat16
    AF = mybir.ActivationFunctionType
    OP = mybir.AluOpType

    # Keep every activation func we use in one table (see baseline comment):
    # Exp, Ln, Square, Copy, Identity all live in natural_log_exp_and_others.
    need = {AF.Exp, AF.Ln, AF.Square, AF.Copy, AF.Identity}
    keep = "natural_log_exp_and_others"
    tabs = hw_specs.get_activation_tables("gen3")
    if keep in tabs and need <= tabs[keep]:
        for name, s in tabs.items():
            if name != keep:
                s -= need

    nch = t // C              # chunks
    nts = t // 512            # 512-wide time slices
    assert t % 512 == 0 and nch % 4 == 0
    ngrp = nch // 4

    nc = bacc.Bacc("TRN2", target_bir_lowering=False, debug=False)

    xt_d = nc.dram_tensor("xt", [128, t, 4], BF16, kind="ExternalInput")
    w1_d = nc.dram_tensor("w1", [128, 4, dk], BF16, kind="ExternalInput")
    w2_d = nc.dram_tensor("w2", [128, 4, 2 * C + dv], BF16, kind="ExternalInput")
    um_d = nc.dram_tensor("umask", [C, C], F32, kind="ExternalInput")
    id_d = nc.dram_tensor("ident", [dk, dk], BF16, kind="ExternalInput")
    wf_d = nc.dram_tensor("wfused", [dv, 10], BF16, kind="ExternalInput")
    nb_d = nc.dram_tensor("nbgk2", [dk, 1], F32, kind="ExternalInput")
    out_d = nc.dram_tensor("out10", [128, nch, 10], F32, kind="ExternalOutput")

    with tile.TileContext(nc) as tc:
        with (
            tc.tile_pool(name="wt", bufs=1) as wt,
            tc.tile_pool(name="sm", bufs=2) as sm,
            tc.tile_pool(name="ck", bufs=3) as ck,
            tc.tile_pool(name="am", bufs=8) as am,
            tc.tile_pool(name="pp", bufs=4, space="PSUM") as pp,
            tc.tile_pool(name="pq", bufs=1, space="PSUM") as pq,
        ):
            # ---- persistent SBUF ----
            xt = wt.tile([128, t, 4], BF16)
            w1 = wt.tile([128, 4, dk], BF16)           # wgk (low-rank fused)
            w2 = wt.tile([128, 4, 2 * C + dv], BF16)   # [wqk 128 | wg 128 | wv 128]
            um_sb = wt.tile([C, C], F32)
            id_sb = wt.tile([dk, dk], BF16)
            wf_sb = wt.tile([dv, 10], BF16)
            nb_sb = wt.tile([dk, 1], F32)

            # input DMAs: weights ahead of the x stream on the sync queue.
            # first 512-slice split into 4 pieces so slice-0 projections can
            # start on partial data.
            nc.sync.dma_start(w1[:], w1_d[:])
            for p in range(4):
                nc.sync.dma_start(xt[:, p * C:(p + 1) * C, :],
                                  xt_d[:, p * C:(p + 1) * C, :])
            nc.sync.dma_start(w2[:], w2_d[:])
            for j in range(1, nts):
                nc.sync.dma_start(xt[:, j * 512:(j + 1) * 512, :],
                                  xt_d[:, j * 512:(j + 1) * 512, :])
            nc.gpsimd.dma_start(um_sb[:], um_d[:])
            nc.gpsimd.dma_start(id_sb[:], id_d[:])
            nc.gpsimd.dma_start(wf_sb[:], wf_d[:])
            nc.gpsimd.dma_start(nb_sb[:], nb_d[:])

            wqk = w2[:, :, 0:C]
            wg = w2[:, :, C:2 * C]
            wv = w2[:, :, 2 * C:2 * C + dv]

            ones64 = wt.tile([dk, 1], F32)
            nc.vector.memset(ones64[:], 1.0)
            onesbf = wt.tile([128, 1], BF16)
            nc.vector.memset(onesbf[:], 1.0)
            eps_sb = wt.tile([128, 1], F32)
            nc.vector.memset(eps_sb[:], EPS)
            # scan reset mask: 0 at chunk starts
            mres = wt.tile([dk, 512], F32)
            nc.vector.memset(mres[:], 1.0)
            mres_v = mres[:].rearrange("p (c l) -> p c l", l=C)
            nc.vector.memset(mres_v[:, :, 0:1], 0.0)

            # big SBUF activations
            spc = wt.tile([dk, t], F32)       # per-chunk cumsum of softplus
            qt_t = wt.tile([dk, t], BF16)     # q~^T
            kt_t = wt.tile([dk, t], BF16)     # k~^T
            swt = wt.tile([dv, t], BF16)      # swish(g)^T
            vsb = wt.tile([128, nch, dv], BF16)
            dlast = wt.tile([dk, nch], F32)
            ktn = wt.tile([C, nch, dk], BF16)
            Sb = wt.tile([dk, nch, dv], BF16)
            obuf = wt.tile([128, nch, 10], F32)

            dl_src = spc[:].rearrange("p (c l) -> p c l", l=C)

            # ---- PSUM (bank-granular: 4 + 1 + 1 + 1 = 7 of 8 banks) ----
            prs = pq.tile([128, 512], F32)    # pat 2 slots | pds 2 slots
            pvt = pq.tile([128, 4, dv], F32)  # v projection, 4 rotating slots
            pot = pq.tile([128, 4, C], F32)   # po^T, 4 rotating slots
            # ssq cols | p10 cols | 2 pkt slots (bf16-bitcast, 32 f32 cols ea)
            pt2 = pq.tile([128, nch + nch * 10 + dk], F32)

            def pat_s(c):
                return prs[:, (c % 2) * C:(c % 2) * C + C]

            def pds_s(c):
                return prs[0:dk, 256 + (c % 2) * C:256 + (c % 2) * C + C]

            def pkt_s(c):
                base = nch + nch * 10 + (c % 2) * (dk // 2)
                return pt2[:, base:base + dk // 2].bitcast(BF16)

            def ssq_s(c):
                return pt2[:, c:c + 1]

            def p10_s(c):
                return pt2[:, nch + c * 10:nch + (c + 1) * 10]

            # ---------------- phase A: projections ----------------
            for j in range(nts):
                ts = slice(j * 512, (j + 1) * 512)
                xs = xt[:, ts, :]
                # j=0 runs piecewise so matmuls start as soon as the first
                # 128-col DMA piece lands
                pieces = ([slice(p * C, (p + 1) * C) for p in range(4)]
                          if j == 0 else [slice(0, 512)])

                def proj(ps, w_sb):
                    for pr in pieces:
                        for d4 in range(4):
                            nc.tensor.matmul(ps[:, pr], w_sb[:, d4, :],
                                             xs[:, pr, d4],
                                             start=(d4 == 0), stop=(d4 == 3))

                # gate chain
                pz = pp.tile([dk, 512], F32, tag="P")
                proj(pz, w1)
                eg = sm.tile([dk, 512], BF16, tag="eg")
                nc.scalar.activation(out=eg[:], in_=pz[:], func=AF.Exp,
                                     scale=-1.0, bias=nb_sb[:])
                sp = sm.tile([dk, 512], F32, tag="sp")
                nc.scalar.activation(out=sp[:], in_=eg[:], func=AF.Ln,
                                     bias=ones64[:])
                nc.vector.tensor_tensor_scan(
                    out=spc[:, ts], data0=mres[:], data1=sp[:],
                    initial=0.0, op0=OP.mult, op1=OP.add)
                nc.scalar.activation(out=dlast[:, 4 * j:4 * j + 4],
                                     in_=dl_src[:, 4 * j:4 * j + 4, C - 1:C],
                                     func=AF.Exp, scale=-1.0 / GATE_NORM)
                eeq = sm.tile([dk, 512], F32, tag="eeq")
                nc.scalar.activation(out=eeq[:], in_=spc[:, ts], func=AF.Exp,
                                     scale=-1.0 / GATE_NORM)
                eek = sm.tile([dk, 512], F32, tag="eek")
                nc.scalar.activation(out=eek[:], in_=spc[:, ts], func=AF.Exp,
                                     scale=1.0 / GATE_NORM)

                # q|k projection + decay
                pqk = pp.tile([128, 512], F32, tag="P")
                proj(pqk, wqk)
                nc.vector.tensor_tensor(out=qt_t[:, ts], in0=pqk[0:dk, :],
                                        in1=eeq[:], op=OP.mult)
                nc.vector.tensor_tensor(out=kt_t[:, ts], in0=pqk[64:128, :],
                                        in1=eek[:], op=OP.mult)

                # g^T projection + swish
                pgt = pp.tile([128, 512], F32, tag="P")
                proj(pgt, wg)
                eg2 = sm.tile([dv, 512], BF16, tag="eg2")
                nc.scalar.activation(out=eg2[:], in_=pgt[:], func=AF.Exp,
                                     scale=-1.0)
                s1 = sm.tile([dv, 512], F32, tag="s1")
                nc.vector.tensor_scalar_add(out=s1[:], in0=eg2[:], scalar1=1.0)
                s2 = sm.tile([dv, 512], F32, tag="s2")
                nc.vector.reciprocal_approx_fast(out=s2[:], in_=s1[:])
                nc.vector.tensor_tensor(out=swt[:, ts], in0=pgt[:], in1=s2[:],
                                        op=OP.mult)

                # v natural projections
                for i in range(4):
                    tt = 4 * j + i
                    pvs = pvt[:, tt % 4, :]
                    for d4 in range(4):
                        nc.tensor.matmul(pvs, xs[:, i * C:(i + 1) * C, d4],
                                         wv[:, d4, :],
                                         start=(d4 == 0), stop=(d4 == 3))
                    if i % 2 == 0:
                        nc.scalar.copy(vsb[:, tt, :], pvs)
                    else:
                        nc.vector.tensor_copy(vsb[:, tt, :], pvs)

            # ---------------- phase B/C: chunked recurrence ----------------
            U = [None, None]

            def emit_c(c):
                """post-processing of chunk c (po complete)."""
                po = pot[:, c % 4, :]
                ot = ck.tile([dv, C], BF16, tag="ot")
                nc.vector.tensor_tensor(out=ot[:], in0=po, in1=swt[:, sl(c)],
                                        op=OP.mult)
                sq = ck.tile([dv, C], BF16, tag="sq")
                nc.scalar.activation(out=sq[:], in_=po, func=AF.Square)
                nc.tensor.matmul(ssq_s(c), sq[:], onesbf[:],
                                 start=True, stop=True)
                nc.tensor.matmul(p10_s(c), ot[:], wf_sb[:],
                                 start=True, stop=True)
                if c % 4 == 3:
                    g = c // 4
                    lnv = ck.tile([128, 4], F32, tag="lnv")
                    nc.scalar.activation(out=lnv[:], in_=pt2[:, 4 * g:4 * g + 4],
                                         func=AF.Ln, scale=1.0 / dv,
                                         bias=eps_sb[:])
                    rstd = ck.tile([128, 4], F32, tag="rstd")
                    nc.scalar.activation(out=rstd[:], in_=lnv[:], func=AF.Exp,
                                         scale=-0.5)
                    p10g = pt2[:, nch + g * 40:nch + (g + 1) * 40]
                    nc.vector.tensor_tensor(
                        out=obuf[:, 4 * g:4 * g + 4, :],
                        in0=p10g.rearrange("p (c n) -> p c n", n=10),
                        in1=rstd[:].unsqueeze(2).broadcast_to([128, 4, 10]),
                        op=OP.mult)
                    nc.sync.dma_start(out_d[:, 4 * g:4 * g + 4, :],
                                      obuf[:, 4 * g:4 * g + 4, :])

            def sl(c):
                return slice(c * C, (c + 1) * C)

            # Software pipeline with lag LAG: state-independent work (pat/atm/
            # ktn/pds + scan) runs ahead; both po matmuls (state part opens
            # the psum group, intra part closes it) and the chunk post-
            # processing trail LAG chunks behind, so the serial scan chain
            # never stalls the PE stream.  Only one po group open at a time.
            LAG = min(6, nch - 1)
            atms = [None] * nch

            def emit_tail(x):
                po = pot[:, x % 4, :]
                if x > 0:
                    nc.tensor.matmul(po, Sb[:, x - 1, :], qt_t[:, sl(x)],
                                     start=True, stop=False)
                nc.tensor.matmul(po, vsb[:, x, :], atms[x][:],
                                 start=(x == 0), stop=True)
                emit_c(x)

            for c in range(nch):
                cs = sl(c)
                qt_c = qt_t[:, cs]
                kt_c = kt_t[:, cs]
                v_c = vsb[:, c, :]
                # B1
                pat = pat_s(c)
                nc.tensor.matmul(pat, kt_c, qt_c, start=True, stop=True)
                atm = am.tile([C, C], BF16, tag="atm")
                atms[c] = atm
                nc.vector.tensor_tensor(out=atm[:], in0=pat, in1=um_sb[:],
                                        op=OP.mult)
                pkt = pkt_s(c)
                nc.tensor.transpose(pkt, kt_c, id_sb[:])
                nc.scalar.copy(ktn[:, c, :], pkt)
                pds = pds_s(c)
                nc.tensor.matmul(pds, ktn[:, c, :], v_c, start=True, stop=True)
                # B2: U_c = U_{c-1} * d_{c-1} + pds_c ; Sb_c = bf16(U_c * d_c)
                Uc = ck.tile([dk, dv], F32, tag="U")
                if c == 0:
                    nc.vector.tensor_copy(Uc[:], pds)
                else:
                    nc.vector.scalar_tensor_tensor(
                        out=Uc[:], in0=U[(c - 1) % 2][:],
                        scalar=dlast[:, c - 1:c], op0=OP.mult,
                        in1=pds, op1=OP.add)
                U[c % 2] = Uc
                nc.gpsimd.tensor_tensor(
                    out=Sb[:, c, :], in0=Uc[:],
                    in1=dlast[:, c:c + 1].broadcast_to([dk, dv]), op=OP.mult)
                if c >= LAG:
                    emit_tail(c - LAG)
            for x in range(nch - LAG, nch):
                emit_tail(x)

    nc.compile()
    return nc


def _prep_inputs(inputs, t=T):
    """Per-core input dicts: core = 4*b + h."""
    import ml_dtypes
    bf16 = ml_dtypes.bfloat16
    ins = {k: np.ascontiguousarray(np.asarray(v, dtype=np.float32))
           for k, v in inputs.items()}
    x, Wq, Wk, Wv, Wg = ins["x"], ins["Wq"], ins["Wk"], ins["Wv"], ins["Wg"]
    Wgk12 = (ins["Wgk1"].astype(np.float64) @ ins["Wgk2"].astype(np.float64))
    bgk2, gnorm = ins["bgk2"], ins["gnorm_w"]
    Wo, Whead = ins["Wo"], ins["Whead"]

    um = (np.arange(C)[:, None] <= np.arange(C)[None, :]).astype(np.float32)
    ident = np.eye(dk, dtype=np.float32).astype(bf16)

    def chunk_w(w):  # [512, n] -> [128, 4, n]
        return np.ascontiguousarray(
            w.reshape(4, 128, -1).transpose(1, 0, 2).astype(bf16))

    in_maps = []
    for core in range(8):
        b, h = divmod(core, 4)
        wf = ((gnorm[:, None].astype(np.float64)
               * Wo[h * dv:(h + 1) * dv, :].astype(np.float64))
              @ Whead.astype(np.float64)).astype(np.float32)
        w2 = np.concatenate(
            [Wq[:, h * dk:(h + 1) * dk] * SCALE, Wk[:, h * dk:(h + 1) * dk],
             Wg[:, h * dv:(h + 1) * dv], Wv[:, h * dv:(h + 1) * dv]], axis=1)
        in_maps.append({
            "xt": np.ascontiguousarray(
                x[b, :t].reshape(t, 4, 128).transpose(2, 0, 1).astype(bf16)),
            "w1": chunk_w(Wgk12[:, h * dk:(h + 1) * dk].astype(np.float32)),
            "w2": chunk_w(w2),
            "umask": um,
            "ident": np.ascontiguousarray(ident),
            "wfused": np.ascontiguousarray(wf.astype(bf16)),
            "nbgk2": np.ascontiguousarray(-bgk2[h * dk:(h + 1) * dk, None]),
        })
    return in_maps


def _gather(results, inputs, t=T):
    bhead = np.asarray(inputs["bhead"], dtype=np.float32)
    out = np.zeros((B, t, 10), np.float32)
    for core in range(8):
        b = core // 4
        r = results[core]["out10"]          # [128, nch, 10]
        out[b] += r.transpose(1, 0, 2).reshape(t, 10)
    out += bhead[None, None, :]
    return out


def run(inputs, trace=False, **kw):
    from concourse.bass_utils import run_bass_kernel_spmd
    if "nc" not in _CACHE:
        _CACHE["nc"] = build()
    nc = _CACHE["nc"]
    in_maps = _prep_inputs(inputs)
    res = run_bass_kernel_spmd(nc, in_maps, core_ids=list(range(8)),
                               trace=trace, **kw)
    return _gather(res.results, inputs), res


def kernel(**inputs) -> np.ndarray:
    out, _ = run(inputs, trace=False)
    return out


# revision 32
# speedup vs baseline: 2.1138x; 1.0764x over previous
"""Gated linear attention (GLA) Bass kernel for Trainium2, 8 NeuronCores.

Sharding: one core per (batch, head) pair -- B=2 x H=4 = 8 cores.

v2 redesign vs baseline (145us):
  - bf16 throughout: x, weights, chunk matmuls (4x PE rate at 128-col moving
    dim; FWL weight loads; half the HBM traffic for x).
  - transposed-o formulation: po^T[dv,t] = v^T atm + S^T q~ accumulates both
    recurrence terms in one psum group, killing the per-chunk output
    transpose + copy of the baseline.
  - phases pipelined across chunks instead of a serial per-chunk chain:
    B1 (pat/ktn/pds/po-intra, independent per chunk), B2 (state scan
    U_c = U_{c-1}*d_{c-1} + pds_c as one fused vector op reading psum),
    B3 (po += S^T q~), C (rmsnorm via matmul-with-ones + batched Ln/Exp).
  - rmsnorm rstd applied on the 10-col head output instead of the dv-wide
    tensor; sum-of-squares via PE matmul against ones.
  - decay exp computed as ONE [128,512] activation (q|k halves stacked with
    pre-negated sign); softmax scale folded into Wq on host.
  - swish split across scalar/gpsimd/vector; cumsum j>0 on gpsimd.
  - x streamed t-major [128,T,4] so time slices are contiguous DMAs;
    weights packed into 2 DMAs ordered ahead of the x stream.
"""
import sys, os
sys.path.insert(0, "/opt/trn_rl_repo")

import numpy as np

B, T, D = 2, 2048, 512
H = 4
dk, dv = 64, 128          # per-head key/value dims
C = 128                   # chunk length
GATE_NORM = 16.0
EPS = 1e-5
SCALE = dk ** -0.5

_CACHE = {}


def build(t=T):
    import concourse.bass as bass  # noqa: F401
    from concourse import bacc, mybir
    import concourse.tile as tile
    import concourse.hw_specs as hw_specs

    F32 = mybir.dt.float32
    BF16 = mybir.dt.bfloat16
    AF = mybir.ActivationFunctionType
    OP = mybir.AluOpType

    # Keep every activation func we use in one table (see baseline comment):
    # Exp, Ln, Square, Copy, Identity all live in natural_log_exp_and_others.
    need = {AF.Exp, AF.Ln, AF.Square, AF.Copy, AF.Identity}
    keep = "natural_log_exp_and_others"
    tabs = hw_specs.get_activation_tables("gen3")
    if keep in tabs and need <= tabs[keep]:
        for name, s in tabs.items():
            if name != keep:
                s -= need

    nch = t // C              # chunks
    nts = t // 512            # 512-wide time slices
    assert t % 512 == 0 and nch % 4 == 0
    ngrp = nch // 4

    nc = bacc.Bacc("TRN2", target_bir_lowering=False, debug=False)

    xt_d = nc.dram_tensor("xt", [128, 4, t], BF16, kind="ExternalInput")
    w1_d = nc.dram_tensor("w1", [128, 4, dk], BF16, kind="ExternalInput")
    w2_d = nc.dram_tensor("w2", [128, 4, 2 * C + dv], BF16, kind="ExternalInput")
    um_d = nc.dram_tensor("umask", [C, C], F32, kind="ExternalInput")
    id_d = nc.dram_tensor("ident", [dk, dk], BF16, kind="ExternalInput")
    wf_d = nc.dram_tensor("wfused", [dv, 10], BF16, kind="ExternalInput")
    nb_d = nc.dram_tensor("nbgk2", [dk, 1], F32, kind="ExternalInput")
    out_d = nc.dram_tensor("out10", [128, nch, 10], F32, kind="ExternalOutput")

    with tile.TileContext(nc) as tc:
        with (
            tc.tile_pool(name="wt", bufs=1) as wt,
            tc.tile_pool(name="sm", bufs=2) as sm,
            tc.tile_pool(name="ck", bufs=3) as ck,
            tc.tile_pool(name="am", bufs=8) as am,
            tc.tile_pool(name="pp", bufs=4, space="PSUM") as pp,
            tc.tile_pool(name="pq", bufs=1, space="PSUM") as pq,
        ):
            # ---- persistent SBUF ----
            xt = wt.tile([128, 4, t], BF16)
            w1 = wt.tile([128, 4, dk], BF16)           # wgk (low-rank fused)
            w2 = wt.tile([128, 4, 2 * C + dv], BF16)   # [wqk 128 | wg 128 | wv 128]
            um_sb = wt.tile([C, C], F32)
            id_sb = wt.tile([dk, dk], BF16)
            wf_sb = wt.tile([dv, 10], BF16)
            nb_sb = wt.tile([dk, 1], F32)

            # input DMAs: weights ahead of the x stream on the sync queue.
            # first 512-slice split into 4 pieces so slice-0 projections can
            # start on partial data.
            nc.sync.dma_start(w1[:], w1_d[:])
            for p in range(2):
                nc.sync.dma_start(xt[:, :, p * 256:(p + 1) * 256],
                                  xt_d[:, :, p * 256:(p + 1) * 256])
            nc.sync.dma_start(w2[:], w2_d[:])
            for j in range(1, nts):
                nc.sync.dma_start(xt[:, :, j * 512:(j + 1) * 512],
                                  xt_d[:, :, j * 512:(j + 1) * 512])
            nc.gpsimd.dma_start(um_sb[:], um_d[:])
            nc.gpsimd.dma_start(id_sb[:], id_d[:])
            nc.gpsimd.dma_start(wf_sb[:], wf_d[:])
            nc.gpsimd.dma_start(nb_sb[:], nb_d[:])

            wqk = w2[:, :, 0:C]
            wg = w2[:, :, C:2 * C]
            wv = w2[:, :, 2 * C:2 * C + dv]

            ones64 = wt.tile([dk, 1], F32)
            nc.vector.memset(ones64[:], 1.0)
            onesbf = wt.tile([128, 1], BF16)
            nc.vector.memset(onesbf[:], 1.0)
            eps_sb = wt.tile([128, 1], F32)
            nc.vector.memset(eps_sb[:], EPS)
            # scan reset mask: 0 at chunk starts
            mres = wt.tile([dk, 512], F32)
            nc.vector.memset(mres[:], 1.0)
            mres_v = mres[:].rearrange("p (c l) -> p c l", l=C)
            nc.vector.memset(mres_v[:, :, 0:1], 0.0)

            # big SBUF activations
            spc = wt.tile([dk, t], F32)       # per-chunk cumsum of softplus
            qt_t = wt.tile([dk, t], BF16)     # q~^T
            kt_t = wt.tile([dk, t], BF16)     # k~^T
            swt = wt.tile([dv, t], BF16)      # swish(g)^T
            vsb = wt.tile([128, nch, dv], BF16)
            dlast = wt.tile([dk, nch], F32)
            ktn = wt.tile([C, nch, dk], BF16)
            Sb = wt.tile([dk, nch, dv], BF16)
            obuf = wt.tile([128, nch, 10], F32)

            dl_src = spc[:].rearrange("p (c l) -> p c l", l=C)

            # ---- PSUM (bank-granular: 4 + 1 + 1 + 1 = 7 of 8 banks) ----
            prs = pq.tile([128, 512], F32)    # pat 2 slots | pds 2 slots
            pvt = pq.tile([128, 4, dv], F32)  # v projection, 4 rotating slots
            pot = pq.tile([128, 4, C], F32)   # po^T, 4 rotating slots
            # ssq cols | p10 cols | 2 pkt slots (bf16-bitcast, 32 f32 cols ea)
            pt2 = pq.tile([128, nch + nch * 10 + dk], F32)

            def pat_s(c):
                return prs[:, (c % 2) * C:(c % 2) * C + C]

            def pds_s(c):
                return prs[0:dk, 256 + (c % 2) * C:256 + (c % 2) * C + C]

            def pkt_s(c):
                base = nch + nch * 10 + (c % 2) * (dk // 2)
                return pt2[:, base:base + dk // 2].bitcast(BF16)

            def ssq_s(c):
                return pt2[:, c:c + 1]

            def p10_s(c):
                return pt2[:, nch + c * 10:nch + (c + 1) * 10]

            # ---------------- phase A: projections ----------------
            for j in range(nts):
                ts = slice(j * 512, (j + 1) * 512)
                xs = xt[:, :, ts]
                # j=0 runs piecewise so matmuls start as soon as the first
                # 256-col DMA piece lands
                pieces = ([slice(p * 256, (p + 1) * 256) for p in range(2)]
                          if j == 0 else [slice(0, 512)])

                def proj(ps, w_sb):
                    for pr in pieces:
                        for d4 in range(4):
                            nc.tensor.matmul(ps[:, pr], w_sb[:, d4, :],
                                             xs[:, d4, pr],
                                             start=(d4 == 0), stop=(d4 == 3))

                # gate chain
                pz = pp.tile([dk, 512], F32, tag="P")
                proj(pz, w1)
                eg = sm.tile([dk, 512], BF16, tag="eg")
                nc.scalar.activation(out=eg[:], in_=pz[:], func=AF.Exp,
                                     scale=-1.0, bias=nb_sb[:])
                sp = sm.tile([dk, 512], F32, tag="sp")
                nc.scalar.activation(out=sp[:], in_=eg[:], func=AF.Ln,
                                     bias=ones64[:])
                nc.vector.tensor_tensor_scan(
                    out=spc[:, ts], data0=mres[:], data1=sp[:],
                    initial=0.0, op0=OP.mult, op1=OP.add)
                nc.scalar.activation(out=dlast[:, 4 * j:4 * j + 4],
                                     in_=dl_src[:, 4 * j:4 * j + 4, C - 1:C],
                                     func=AF.Exp, scale=-1.0 / GATE_NORM)
                eeq = sm.tile([dk, 512], F32, tag="eeq")
                nc.scalar.activation(out=eeq[:], in_=spc[:, ts], func=AF.Exp,
                                     scale=-1.0 / GATE_NORM)
                eek = sm.tile([dk, 512], F32, tag="eek")
                nc.scalar.activation(out=eek[:], in_=spc[:, ts], func=AF.Exp,
                                     scale=1.0 / GATE_NORM)

                # q|k projection + decay
                pqk = pp.tile([128, 512], F32, tag="P")
                proj(pqk, wqk)
                nc.vector.tensor_tensor(out=qt_t[:, ts], in0=pqk[0:dk, :],
                                        in1=eeq[:], op=OP.mult)
                nc.vector.tensor_tensor(out=kt_t[:, ts], in0=pqk[64:128, :],
                                        in1=eek[:], op=OP.mult)

                # g^T projection + swish
                pgt = pp.tile([128, 512], F32, tag="P")
                proj(pgt, wg)
                eg2 = sm.tile([dv, 512], BF16, tag="eg2")
                nc.scalar.activation(out=eg2[:], in_=pgt[:], func=AF.Exp,
                                     scale=-1.0)
                s1 = sm.tile([dv, 512], F32, tag="s1")
                nc.vector.tensor_scalar_add(out=s1[:], in0=eg2[:], scalar1=1.0)
                s2 = sm.tile([dv, 512], F32, tag="s2")
                nc.vector.reciprocal_approx_fast(out=s2[:], in_=s1[:])
                nc.vector.tensor_tensor(out=swt[:, ts], in0=pgt[:], in1=s2[:],
                                        op=OP.mult)

                # v natural projections
                for i in range(4):
                    tt = 4 * j + i
                    pvs = pvt[:, tt % 4, :]
                    for d4 in range(4):
                        nc.tensor.matmul(pvs, xs[:, d4, i * C:(i + 1) * C],
                                         wv[:, d4, :],
                                         start=(d4 == 0), stop=(d4 == 3))
                    if i % 2 == 0:
                        nc.scalar.copy(vsb[:, tt, :], pvs)
                    else:
                        nc.vector.tensor_copy(vsb[:, tt, :], pvs)

            # ---------------- phase B/C: chunked recurrence ----------------
            U = [None, None]

            def emit_c(c):
                """post-processing of chunk c (po complete)."""
                po = pot[:, c % 4, :]
                ot = ck.tile([dv, C], BF16, tag="ot")
                nc.vector.tensor_tensor(out=ot[:], in0=po, in1=swt[:, sl(c)],
                                        op=OP.mult)
                sq = ck.tile([dv, C], BF16, tag="sq")
                nc.scalar.activation(out=sq[:], in_=po, func=AF.Square)
                nc.tensor.matmul(ssq_s(c), sq[:], onesbf[:],
                                 start=True, stop=True)
                nc.tensor.matmul(p10_s(c), ot[:], wf_sb[:],
                                 start=True, stop=True)
                if c % 4 == 3:
                    g = c // 4
                    lnv = ck.tile([128, 4], F32, tag="lnv")
                    nc.scalar.activation(out=lnv[:], in_=pt2[:, 4 * g:4 * g + 4],
                                         func=AF.Ln, scale=1.0 / dv,
                                         bias=eps_sb[:])
                    rstd = ck.tile([128, 4], F32, tag="rstd")
                    nc.scalar.activation(out=rstd[:], in_=lnv[:], func=AF.Exp,
                                         scale=-0.5)
                    p10g = pt2[:, nch + g * 40:nch + (g + 1) * 40]
                    nc.vector.tensor_tensor(
                        out=obuf[:, 4 * g:4 * g + 4, :],
                        in0=p10g.rearrange("p (c n) -> p c n", n=10),
                        in1=rstd[:].unsqueeze(2).broadcast_to([128, 4, 10]),
                        op=OP.mult)
                    nc.sync.dma_start(out_d[:, 4 * g:4 * g + 4, :],
                                      obuf[:, 4 * g:4 * g + 4, :])

            def sl(c):
                return slice(c * C, (c + 1) * C)

            # Software pipeline with lag LAG: state-independent work (pat/atm/
            # ktn/pds + scan) runs ahead; both po matmuls (state part opens
            # the psum group, intra part closes it) and the chunk post-
            # processing trail LAG chunks behind, so the serial scan chain
            # never stalls the PE stream.  Only one po group open at a time.
            LAG = min(6, nch - 1)
            atms = [None] * nch

            def emit_tail(x):
                po = pot[:, x % 4, :]
                if x > 0:
                    nc.tensor.matmul(po, Sb[:, x - 1, :], qt_t[:, sl(x)],
                                     start=True, stop=False)
                nc.tensor.matmul(po, vsb[:, x, :], atms[x][:],
                                 start=(x == 0), stop=True)
                emit_c(x)

            for c in range(nch):
                cs = sl(c)
                qt_c = qt_t[:, cs]
                kt_c = kt_t[:, cs]
                v_c = vsb[:, c, :]
                # B1
                pat = pat_s(c)
                nc.tensor.matmul(pat, kt_c, qt_c, start=True, stop=True)
                atm = am.tile([C, C], BF16, tag="atm")
                atms[c] = atm
                nc.vector.tensor_tensor(out=atm[:], in0=pat, in1=um_sb[:],
                                        op=OP.mult)
                pkt = pkt_s(c)
                nc.tensor.transpose(pkt, kt_c, id_sb[:])
                nc.scalar.copy(ktn[:, c, :], pkt)
                pds = pds_s(c)
                nc.tensor.matmul(pds, ktn[:, c, :], v_c, start=True, stop=True)
                # B2: U_c = U_{c-1} * d_{c-1} + pds_c ; Sb_c = bf16(U_c * d_c)
                Uc = ck.tile([dk, dv], F32, tag="U")
                if c == 0:
                    nc.vector.tensor_copy(Uc[:], pds)
                else:
                    nc.vector.scalar_tensor_tensor(
                        out=Uc[:], in0=U[(c - 1) % 2][:],
                        scalar=dlast[:, c - 1:c], op0=OP.mult,
                        in1=pds, op1=OP.add)
                U[c % 2] = Uc
                nc.gpsimd.tensor_tensor(
                    out=Sb[:, c, :], in0=Uc[:],
                    in1=dlast[:, c:c + 1].broadcast_to([dk, dv]), op=OP.mult)
                if c >= LAG:
                    emit_tail(c - LAG)
            for x in range(nch - LAG, nch):
                emit_tail(x)

    nc.compile()
    return nc


def _prep_inputs(inputs, t=T):
    """Per-core input dicts: core = 4*b + h."""
    import ml_dtypes
    bf16 = ml_dtypes.bfloat16
    ins = {k: np.ascontiguousarray(np.asarray(v, dtype=np.float32))
           for k, v in inputs.items()}
    x, Wq, Wk, Wv, Wg = ins["x"], ins["Wq"], ins["Wk"], ins["Wv"], ins["Wg"]
    Wgk12 = (ins["Wgk1"].astype(np.float64) @ ins["Wgk2"].astype(np.float64))
    bgk2, gnorm = ins["bgk2"], ins["gnorm_w"]
    Wo, Whead = ins["Wo"], ins["Whead"]

    um = (np.arange(C)[:, None] <= np.arange(C)[None, :]).astype(np.float32)
    ident = np.eye(dk, dtype=np.float32).astype(bf16)

    def chunk_w(w):  # [512, n] -> [128, 4, n]
        return np.ascontiguousarray(
            w.reshape(4, 128, -1).transpose(1, 0, 2).astype(bf16))

    in_maps = []
    for core in range(8):
        b, h = divmod(core, 4)
        wf = ((gnorm[:, None].astype(np.float64)
               * Wo[h * dv:(h + 1) * dv, :].astype(np.float64))
              @ Whead.astype(np.float64)).astype(np.float32)
        w2 = np.concatenate(
            [Wq[:, h * dk:(h + 1) * dk] * SCALE, Wk[:, h * dk:(h + 1) * dk],
             Wg[:, h * dv:(h + 1) * dv], Wv[:, h * dv:(h + 1) * dv]], axis=1)
        in_maps.append({
            "xt": np.ascontiguousarray(
                x[b, :t].T.reshape(4, 128, t).transpose(1, 0, 2).astype(bf16)),
            "w1": chunk_w(Wgk12[:, h * dk:(h + 1) * dk].astype(np.float32)),
            "w2": chunk_w(w2),
            "umask": um,
            "ident": np.ascontiguousarray(ident),
            "wfused": np.ascontiguousarray(wf.astype(bf16)),
            "nbgk2": np.ascontiguousarray(-bgk2[h * dk:(h + 1) * dk, None]),
        })
    return in_maps


def _gather(results, inputs, t=T):
    bhead = np.asarray(inputs["bhead"], dtype=np.float32)
    out = np.zeros((B, t, 10), np.float32)
    for core in range(8):
        b = core // 4
        r = results[core]["out10"]          # [128, nch, 10]
        out[b] += r.transpose(1, 0, 2).reshape(t, 10)
    out += bhead[None, None, :]
    return out


def run(inputs, trace=False, **kw):
    from concourse.bass_utils import run_bass_kernel_spmd
    if "nc" not in _CACHE:
        _CACHE["nc"] = build()
    nc = _CACHE["nc"]
    in_maps = _prep_inputs(inputs)
    res = run_bass_kernel_spmd(nc, in_maps, core_ids=list(range(8)),
                               trace=trace, **kw)
    return _gather(res.results, inputs), res


def kernel(**inputs) -> np.ndarray:
    out, _ = run(inputs, trace=False)
    return out
